# revision 26
# baseline (speedup 1.0000x reference)
"""Trainium2 Bass kernel for nn_CausalAffineAutoregFlow.

Causal affine autoregressive flow: 32-step scan; each step runs two MLPs
(32->100->100->1, relu/relu/sigmoid) on masked z and writes one column of z.

Strategy (pure data parallel over 8 cores, BC=16384 rows each):
- Feature-major ("transposed") layout on chip: z^T lives as a [128, BC]
  SBUF tile holding 4 replicated copies of z^T (32 rows each) so that the
  four row-strips of the PE array can run 4 concurrent K=32 L1 matmuls
  (tile_position row packing).
- Host folds the causal mask C[:,i] into per-step L1 weights (W1m_i =
  diag(C[:,i]) @ W1), so L1 is a plain matmul of z^T.
- Matmuls run in float32r (full-rate fp32 on the PE; ~1e-4 rounding).
- sigmoid(x) = 0.5 + 0.5*tanh(x/2): tanh and exp share one ACT table set.
- log_det = sum_i s_i = 16 + 0.5 * sum_i tanh_i -> accumulate tanh only,
  final affine on host.
- Per step the [1, BC] logits land partition-scattered (col-packed M=1
  matmuls); DMA repacks them batch-major [128, 128] for the z-update ops,
  and one more DMA scatters the new z row back into the 4 z^T replicas.
"""
import numpy as np

import concourse.bass as bass
import concourse.mybir as mybir
import concourse.tile as tile
from concourse import bacc
from concourse.bass_utils import run_bass_kernel_spmd

f32 = mybir.dt.float32
f32r = mybir.dt.float32r
AFT = mybir.ActivationFunctionType
ALU = mybir.AluOpType

DIM = 32
NH = 100
B = 131072
NCORES = 8
BC = B // NCORES  # 16384


def build(bc=BC, nstep=DIM, dbg=False):
    ngrp = bc // 1024  # groups of 2 pairs = 1024 batch cols
    nchunk = bc // 128  # batch-major chunks
    nc = bacc.Bacc("TRN2", target_bir_lowering=False, debug=False, num_devices=1)

    # ---- DRAM I/O ----
    e_d = nc.dram_tensor("e_bm", [128, nchunk * DIM], f32, kind="ExternalInput")
    w1_d = nc.dram_tensor("w1stack", [128, nstep * NH], f32r, kind="ExternalInput")
    w2s_d = nc.dram_tensor("w2s", [NH, NH], f32r, kind="ExternalInput")
    w2t_d = nc.dram_tensor("w2t", [NH, NH], f32r, kind="ExternalInput")
    w3_d = nc.dram_tensor("w3", [NH, 512], f32r, kind="ExternalInput")
    b1s_d = nc.dram_tensor("b1s", [NH, 1], f32, kind="ExternalInput")
    b1t_d = nc.dram_tensor("b1t", [NH, 1], f32, kind="ExternalInput")
    b2s_d = nc.dram_tensor("b2s", [NH, 1], f32, kind="ExternalInput")
    b2t_d = nc.dram_tensor("b2t", [NH, 1], f32, kind="ExternalInput")
    b3_d = nc.dram_tensor("b3t", [128, 1], f32, kind="ExternalInput")
    z_out_d = nc.dram_tensor("z_out", [DIM, bc], f32, kind="ExternalOutput")
    acc_d = nc.dram_tensor("acc_out", [128, nchunk], f32, kind="ExternalOutput")
    if dbg:
        st_d = nc.dram_tensor("st_out", [128, 512 * ngrp], f32,
                              kind="ExternalOutput")
        u_d = nc.dram_tensor("u_out", [128, nchunk], f32, kind="ExternalOutput")
        v_d = nc.dram_tensor("v_out", [128, nchunk], f32, kind="ExternalOutput")
        h1_d = nc.dram_tensor("h1s_out", [NH, 1024], f32, kind="ExternalOutput")
        h2_d = nc.dram_tensor("h2s_out", [NH, 1024], f32, kind="ExternalOutput")

    with tile.TileContext(nc) as tc:
        with tc.tile_pool(name="big", bufs=1) as big, \
             tc.tile_pool(name="hpool", bufs=2) as hp, \
             tc.tile_pool(name="bm", bufs=2) as bmp, \
             tc.tile_pool(name="psA", bufs=1, space="PSUM") as psA, \
             tc.tile_pool(name="ps3", bufs=2, space="PSUM") as ps3:

            # ---- persistent SBUF ----
            zT4 = big.tile([128, bc], f32r, tag="zT4")
            e_bm = big.tile([128, nchunk * DIM], f32, tag="e_bm")
            w1_s = big.tile([128, nstep * NH], f32r, tag="w1")
            w2s_s = big.tile([NH, NH], f32r, tag="w2s")
            w2t_s = big.tile([NH, NH], f32r, tag="w2t")
            w3_s = big.tile([NH, 512], f32r, tag="w3")
            b1s_s = big.tile([NH, 1], f32, tag="b1s")
            b1t_s = big.tile([NH, 1], f32, tag="b1t")
            b2s_s = big.tile([NH, 1], f32, tag="b2s")
            b2t_s = big.tile([NH, 1], f32, tag="b2t")
            b3_s = big.tile([128, 1], f32, tag="b3")
            half_s = big.tile([128, 1], f32, tag="half")
            st = big.tile([128, 512 * ngrp], f32, tag="st")
            acc = big.tile([128, nchunk], f32, tag="acc")

            nc.sync.dma_start(e_bm[:], e_d.ap())
            nc.sync.dma_start(w1_s[:], w1_d.ap())
            nc.sync.dma_start(w2s_s[:], w2s_d.ap())
            nc.sync.dma_start(w2t_s[:], w2t_d.ap())
            nc.sync.dma_start(w3_s[:], w3_d.ap())
            nc.sync.dma_start(b1s_s[:], b1s_d.ap())
            nc.sync.dma_start(b1t_s[:], b1t_d.ap())
            nc.sync.dma_start(b2s_s[:], b2s_d.ap())
            nc.sync.dma_start(b2t_s[:], b2t_d.ap())
            nc.sync.dma_start(b3_s[:], b3_d.ap())

            nc.vector.memset(zT4[:].bitcast(f32), 0.0)
            nc.vector.memset(half_s[:], 0.5)
            nc.vector.memset(acc[:], 0.0)

            zT4_t = zT4[:].tensor
            st_t = st[:].tensor
            st_pitch = 512 * ngrp

            for i in range(nstep):
                wsl = w1_s[:, NH * i:NH * (i + 1)]
                for g in range(ngrp):
                    ca = 512 * g            # A-half columns
                    cb = bc // 2 + 512 * g  # B-half columns
                    # ---- L1: 4 concurrent row-packed matmuls (K=32, N=512)
                    # L1 and L2 psum share pool slots (p2* reuses p1*'s
                    # banks after the L1 drain frees them)
                    p1s = psA.tile([NH, 1024], f32, tag="pss")
                    p1t = psA.tile([NH, 1024], f32, tag="pst")
                    # strips: 0=s/pairE 1=t/pairE 2=s/pairO 3=t/pairO
                    nc.tensor.matmul(p1s[:, 0:512], wsl[0:32, :],
                                     zT4[0:32, ca:ca + 512],
                                     start=True, stop=True, tile_position=(0, 0))
                    nc.tensor.matmul(p1t[:, 0:512], wsl[32:64, :],
                                     zT4[32:64, ca:ca + 512],
                                     start=True, stop=True, tile_position=(32, 0))
                    nc.tensor.matmul(p1s[:, 512:1024], wsl[64:96, :],
                                     zT4[64:96, cb:cb + 512],
                                     start=True, stop=True, tile_position=(64, 0))
                    nc.tensor.matmul(p1t[:, 512:1024], wsl[96:128, :],
                                     zT4[96:128, cb:cb + 512],
                                     start=True, stop=True, tile_position=(96, 0))
                    # ---- L1 drain: relu(x + b1) ----
                    h1s = hp.tile([NH, 1024], f32r, tag="h1s")
                    h1t = hp.tile([NH, 1024], f32r, tag="h1t")
                    nc.scalar.activation(h1s[:], p1s[:], AFT.Relu, bias=b1s_s[:])
                    nc.vector.tensor_scalar(h1t[:], p1t[:], b1t_s[:], 0.0,
                                            op0=ALU.add, op1=ALU.max)
                    # ---- L2 (K=100, N=512) ----
                    p2s = psA.tile([NH, 1024], f32, tag="pss")
                    p2t = psA.tile([NH, 1024], f32, tag="pst")
                    nc.tensor.matmul(p2s[:, 0:512], w2s_s[:], h1s[:, 0:512],
                                     start=True, stop=True)
                    nc.tensor.matmul(p2s[:, 512:1024], w2s_s[:], h1s[:, 512:1024],
                                     start=True, stop=True)
                    nc.tensor.matmul(p2t[:, 0:512], w2t_s[:], h1t[:, 0:512],
                                     start=True, stop=True)
                    nc.tensor.matmul(p2t[:, 512:1024], w2t_s[:], h1t[:, 512:1024],
                                     start=True, stop=True)
                    # ---- L2 drain ----
                    h2s = hp.tile([NH, 1024], f32r, tag="h2s")
                    h2t = hp.tile([NH, 1024], f32r, tag="h2t")
                    nc.scalar.activation(h2s[:], p2s[:], AFT.Relu, bias=b2s_s[:])
                    nc.vector.tensor_scalar(h2t[:], p2t[:], b2t_s[:], 0.0,
                                            op0=ALU.add, op1=ALU.max)
                    if dbg and i == 0 and g == 0:
                        nc.sync.dma_start(h1_d.ap(), h1s[:].bitcast(f32))
                        nc.sync.dma_start(h2_d.ap(), h2s[:].bitcast(f32))
                    # ---- L3: 4 accumulating M=128 matmuls (K=100, N=512)
                    # w3stack slice k is zero except its 32-row block, so
                    # the accumulation chain fills one [128, 512] bank:
                    # rows 0-31=s/pairE 32-63=s/pairO 64-95=t/pairE
                    # 96-127=t/pairO (each row block is 32 replicas).
                    p3 = ps3.tile([128, 512], f32, tag="p3")
                    nc.tensor.matmul(p3[:], w3_s[:, 0:128], h2s[:, 0:512],
                                     start=True, stop=False)
                    nc.tensor.matmul(p3[:], w3_s[:, 128:256], h2s[:, 512:1024],
                                     start=False, stop=False)
                    nc.tensor.matmul(p3[:], w3_s[:, 256:384], h2t[:, 0:512],
                                     start=False, stop=False)
                    nc.tensor.matmul(p3[:], w3_s[:, 384:512], h2t[:, 512:1024],
                                     start=False, stop=True)
                    # ---- tanh(0.5*logit + 0.5*b3) -> staging ----
                    nc.scalar.activation(st[:, 512 * g:512 * (g + 1)],
                                         p3[:], AFT.Tanh,
                                         bias=b3_s[:], scale=0.5)

                # ---- repack logits batch-major: u (s-branch), v (t-branch)
                # bm layout: batch b = 128*j + p  <->  tile[p, j]
                # (partition-fastest, so the DMAs cross partitions only in
                # the leading dst dim)
                u_bm = bmp.tile([128, nchunk], f32, tag="u_bm")
                v_bm = bmp.tile([128, nchunk], f32, tag="v_bm")
                # Batch<->column permutation: within half pr, column
                # c = p*hc + jl holds batch b = pr*H + 128*jl + p, so the
                # repack to bm layout [p, j=pr*hc+jl] reads contiguous
                # hc-element bursts. src reads replica row (32*pr + q),
                # chunked over dst partition quarters.
                hc = nchunk // 2
                for pr in range(2):
                    for q in range(4):
                        src_dims = [[st_pitch, 1], [hc, 32], [1, hc]]
                        dst_dims = [[nchunk, 32], [1, hc]]
                        soff = (32 * pr + q) * st_pitch + 32 * q * hc
                        doff = 32 * q * nchunk + pr * hc
                        nc.sync.dma_start(
                            bass.AP(u_bm[:].tensor, doff, dst_dims),
                            bass.AP(st_t, soff, src_dims))
                        nc.sync.dma_start(
                            bass.AP(v_bm[:].tensor, doff, dst_dims),
                            bass.AP(st_t, soff + 64 * st_pitch, src_dims))

                # ---- batch-major z update ----
                es = bmp.tile([128, nchunk], f32, tag="es")
                tsig = bmp.tile([128, nchunk], f32, tag="tsig")
                tmp = bmp.tile([128, nchunk], f32, tag="tmp")
                znew = bmp.tile([128, nchunk], f32r, tag="znew")
                nc.scalar.activation(es[:], u_bm[:], AFT.Exp, bias=half_s[:], scale=0.5)
                nc.vector.tensor_add(acc[:], acc[:], u_bm[:])
                nc.vector.tensor_scalar(tsig[:], v_bm[:], 1.0, 0.5,
                                        op0=ALU.add, op1=ALU.mult)
                nc.vector.tensor_mul(tmp[:], es[:], e_bm[:, i::DIM])
                nc.vector.tensor_add(znew[:], tmp[:], tsig[:])

                if dbg and i == 0:
                    nc.sync.dma_start(st_d.ap(), st[:])
                    nc.sync.dma_start(u_d.ap(), u_bm[:])
                    nc.sync.dma_start(v_d.ap(), v_bm[:])

                # ---- scatter new z row into the 4 z^T replicas ----
                # znew[p, pr*hc + jl] -> zT4[32r+i, pr*H + p*hc + jl]
                for r in range(4):
                    for pr in range(2):
                        zsrc = bass.AP(znew[:].tensor, pr * hc,
                                       [[nchunk, 128], [1, hc]])
                        nc.sync.dma_start(
                            bass.AP(zT4_t,
                                    (32 * r + i) * bc + pr * (bc // 2),
                                    [[bc, 1], [hc, 128], [1, hc]]),
                            zsrc)

            nc.sync.dma_start(z_out_d.ap(), zT4[0:DIM, :].bitcast(f32))
            nc.sync.dma_start(acc_d.ap(), acc[:])

    nc.compile()
    return nc


def _prep_core_inputs(e_shard, C, ws, nstep=DIM):
    (s_W1, s_b1, s_W2, s_b2, s_W3, s_b3,
     t_W1, t_b1, t_W2, t_b2, t_W3, t_b3) = ws
    bc = e_shard.shape[0]
    nchunk = bc // 128
    # e_bm[p, DIM*(pr*hc+jl) + i] = e[pr*H + 128*jl + p, i]
    hc = nchunk // 2
    e_bm = np.ascontiguousarray(
        e_shard.reshape(2, hc, 128, DIM).transpose(2, 0, 1, 3).reshape(
            128, nchunk * DIM))
    w1stack = np.zeros((128, nstep * NH), np.float32)
    for i in range(nstep):
        sm = C[:, i:i + 1] * s_W1  # [32, 100]
        tm = C[:, i:i + 1] * t_W1
        blk = np.concatenate([sm, tm, sm, tm], axis=0)  # strips s,t,s,t
        w1stack[:, NH * i:NH * (i + 1)] = blk
    w3 = np.zeros((NH, 512), np.float32)
    w3[:, 0:32] = s_W3[:, 0:1]
    w3[:, 128 + 32:128 + 64] = s_W3[:, 0:1]
    w3[:, 256 + 64:256 + 96] = t_W3[:, 0:1]
    w3[:, 384 + 96:384 + 128] = t_W3[:, 0:1]
    b3t = np.zeros((128, 1), np.float32)
    b3t[0:64, 0] = 0.5 * float(s_b3[0])
    b3t[64:128, 0] = 0.5 * float(t_b3[0])
    return {
        "e_bm": e_bm,
        "w1stack": w1stack,
        "w2s": np.ascontiguousarray(s_W2.astype(np.float32)),
        "w2t": np.ascontiguousarray(t_W2.astype(np.float32)),
        "w3": w3,
        "b1s": s_b1.reshape(NH, 1).astype(np.float32),
        "b1t": t_b1.reshape(NH, 1).astype(np.float32),
        "b2s": s_b2.reshape(NH, 1).astype(np.float32),
        "b2t": t_b2.reshape(NH, 1).astype(np.float32),
        "b3t": b3t,
    }


_BUILD_CACHE = {}


def _get_built(bc, nstep):
    key = (bc, nstep)
    if key not in _BUILD_CACHE:
        _BUILD_CACHE[key] = build(bc, nstep)
    return _BUILD_CACHE[key]


def kernel(e, C, s_W1, s_b1, s_W2, s_b2, s_W3, s_b3,
           t_W1, t_b1, t_W2, t_b2, t_W3, t_b3):
    e = np.asarray(e, np.float32)
    C = np.asarray(C, np.float32)
    ws = [np.asarray(w, np.float32) for w in
          (s_W1, s_b1, s_W2, s_b2, s_W3, s_b3,
           t_W1, t_b1, t_W2, t_b2, t_W3, t_b3)]
    ncores = NCORES
    bc = e.shape[0] // ncores
    nstep = e.shape[1]
    nc = _get_built(bc, nstep)
    in_maps = [_prep_core_inputs(e[c * bc:(c + 1) * bc], C, ws, nstep)
               for c in range(ncores)]
    res = run_bass_kernel_spmd(nc, in_maps, list(range(ncores)))
    global LAST_RESULT
    LAST_RESULT = res
    bcv = bc
    hc = bcv // 256
    zs = []
    lds = []
    for c in range(ncores):
        zo = res.results[c]["z_out"]  # [32, bc], cols (pr, p, jl)
        zperm = zo.reshape(DIM, 2, 128, hc).transpose(0, 1, 3, 2).reshape(
            DIM, bcv)  # cols now (pr, jl, p) = batch order
        zs.append(zperm.T)
        ao = res.results[c]["acc_out"]  # [128, nchunk], (p, pr*hc+jl)
        ld = ao.reshape(128, 2, hc).transpose(1, 2, 0).reshape(-1)
        lds.append(0.5 * ld + 0.5 * nstep)
    z = np.concatenate(zs, axis=0)
    ld = np.concatenate(lds, axis=0)
    return z.astype(np.float32), ld.astype(np.float32)
